# revision 23
# baseline (speedup 1.0000x reference)
"""DNDT (deep neural decision tree) forward kernel for 8 Trainium2 NeuronCores.

Math (per batch row b of 16384):
  h[f,j]   = (x[b,f] * W[j] + bias[f,j]) / t,  W = [1..4], bias = cumsum([0,-sorted_cuts])
  bins     = softmax_j(h)                       # [6, 4]
  leaf     = kron(bins[0], ..., bins[5])        # [4096]
  out[b]   = leaf @ leaf_score                  # [10]

Device algorithm (pure data parallel, 2048 rows/core, batch-major layout
[128 partitions x 16 rows-per-partition], fp16 datapath after the exp):
  * softmax shift g(x) = (x + 3*relu(x))/t keeps every exponent <= 0, so
    E = exp(h - g) never overflows; exp runs on the scalar (ACT) engine.
  * bins are normalized PER FEATURE (En = E / sum_j E) right after the exp:
    all kron products then live in [0,1], which makes the fp16 pipeline safe.
  * leaf is factored 4+2: A = p01 (x) p23 (256-wide, [i, b, a] memory order
    with a host-side S2 row permutation), p45 (16-wide);
      C[b,(c,v)] = A @ S2,  S2[u,(c,v)] = leaf_score[u*16+v, c]   (PE, fp16)
      out[b,c]   = sum_v C[b,c,v] * p45[b,v]            (DVE mult + add tree)
  * A is transposed for the matmul by the DMA XBAR (dma_start_transpose,
    SBUF->SBUF fp16) - no PE transposes, no PSUM bounce copies.  The XBARs
    run on the DMA engines and overlap the DVE front.
  * C is copied out of PSUM by ACT in fp16; the v-contraction is a DVE mult
    plus a packed fp16 add-tree (the only shapes the HW runs in 2x mode).
  * output DMAs issue from ACT so the XBARs (on SP) never block them; the
    input DMA order (cst, then x per-super-chunk, then S2) unblocks the DVE
    front as early as possible.
"""

import numpy as np

import concourse.bass as bass
import concourse.tile as tile
from concourse import bacc, mybir
from concourse.bass_utils import run_bass_kernel_spmd

N_CORES = 8
B = 16384
BC = B // N_CORES          # rows per core = 2048
P = 128                    # partitions
M = BC // P                # rows per partition = 16
NSC = 2                    # super-chunks (pipeline stages)
SCM = M // NSC             # rows per partition per super-chunk = 8
F32 = mybir.dt.float32
F16 = mybir.dt.float16
AX = mybir.AxisListType
OP = mybir.AluOpType


def _build_nc(invt):
    neg3invt = -3.0 * invt
    nc = bacc.Bacc("TRN2", target_bir_lowering=False, debug=False,
                   num_devices=N_CORES)
    xd = nc.dram_tensor("x", [P, M * 6], F32, kind="ExternalInput")
    cstd = nc.dram_tensor("cst", [P, 48], F32, kind="ExternalInput")
    s2d = nc.dram_tensor("s2", [256, 160], F16, kind="ExternalInput")
    od = nc.dram_tensor("o", [P, M * 10], F32, kind="ExternalOutput")

    with tile.TileContext(nc) as tc:
        with tc.tile_pool(name="consts", bufs=1) as consts, \
             tc.tile_pool(name="work", bufs=2) as work, \
             tc.tile_pool(name="atp", bufs=2) as atp, \
             tc.tile_pool(name="ps_c", bufs=4, space="PSUM") as ps_c:
            cst_st = consts.tile([P, 2, 6, 4], F32)
            nc.sync.dma_start(out=cst_st[:].rearrange("p k f j -> p (k f j)"),
                              in_=cstd[:])
            x_st = consts.tile([P, M, 6], F32)
            xdv = xd[:].rearrange("p (i f) -> p i f", i=M)
            for sc in range(NSC):
                sl = slice(sc * SCM, (sc + 1) * SCM)
                nc.sync.dma_start(out=x_st[:, sl], in_=xdv[:, sl])
            s2_sb = consts.tile([P, 2, 160], F16)
            nc.sync.dma_start(out=s2_sb[:],
                              in_=s2d[:].rearrange("(k p) n -> p k n", p=P))

            # ---- per-super-chunk front: x -> En (normalized bins, fp16) ----
            # split pre-exp / post-exp so both SCs' H chains queue first and
            # fill the DVE bubble while ACT runs the exps; SC0's post-exp and
            # A then complete as early as possible to start the XBAR chain.
            def front_pre(sc):
                xv = x_st[:, sc * SCM:(sc + 1) * SCM, :]
                r2 = work.tile([P, SCM, 6, 1], F32, tag="r2")
                nc.vector.tensor_scalar(out=r2[:, :, :, 0], in0=xv,
                                        scalar1=0.0, scalar2=neg3invt,
                                        op0=OP.max, op1=OP.mult)
                Hm = work.tile([P, SCM, 6, 4], F32, tag="Hm")
                nc.vector.tensor_mul(
                    Hm[:], xv[:, :, :, None].broadcast_to((P, SCM, 6, 4)),
                    cst_st[:, 0:1, :, :].broadcast_to((P, SCM, 6, 4)))
                G = work.tile([P, SCM, 6, 4], F32, tag="G")
                nc.vector.tensor_add(
                    G[:], r2[:].broadcast_to((P, SCM, 6, 4)),
                    cst_st[:, 1:2, :, :].broadcast_to((P, SCM, 6, 4)))
                H = work.tile([P, SCM, 6, 4], F32, tag="H")
                nc.vector.tensor_add(H[:], Hm[:], G[:])
                E = work.tile([P, SCM, 6, 4], F32, tag="E")
                nc.scalar.activation(E[:].rearrange("p i f j -> p (i f j)"),
                                     H[:].rearrange("p i f j -> p (i f j)"),
                                     mybir.ActivationFunctionType.Exp)
                return E

            def front(sc, E):
                Z = work.tile([P, SCM, 6], F32, tag="Z")
                nc.vector.tensor_reduce(Z[:], E[:], axis=AX.X, op=OP.add)
                zrf = work.tile([P, SCM, 6], F32, tag="zrf")
                nc.vector.reciprocal(zrf[:], Z[:])
                En = work.tile([P, SCM, 6, 4], F16, tag="En")
                nc.vector.tensor_mul(
                    En[:], E[:], zrf[:, :, :, None].broadcast_to((P, SCM, 6, 4)))

                p01 = work.tile([P, SCM, 16], F16, tag="p01")
                nc.vector.tensor_mul(
                    p01[:].rearrange("p i (a b) -> p i a b", a=4),
                    En[:, :, 0, :, None].broadcast_to((P, SCM, 4, 4)),
                    En[:, :, 1, None, :].broadcast_to((P, SCM, 4, 4)))
                p23 = work.tile([P, SCM, 16], F16, tag="p23")
                nc.vector.tensor_mul(
                    p23[:].rearrange("p i (a b) -> p i a b", a=4),
                    En[:, :, 2, :, None].broadcast_to((P, SCM, 4, 4)),
                    En[:, :, 3, None, :].broadcast_to((P, SCM, 4, 4)))
                p45 = work.tile([P, SCM, 16], F16, tag="p45")
                nc.vector.tensor_mul(
                    p45[:].rearrange("p i (a b) -> p i a b", a=4),
                    En[:, :, 4, :, None].broadcast_to((P, SCM, 4, 4)),
                    En[:, :, 5, None, :].broadcast_to((P, SCM, 4, 4)))

                return p01, p23, p45

            # A[i, b, a] = p01[i, a] * p23[i, b]: two DVE instructions
            # aligned with the XBAR halves (single producer each).
            def build_a(p01, p23):
                A = work.tile([P, SCM, 16, 16], F16, tag="A")
                for sl in (slice(0, 4), slice(4, 8)):
                    n = sl.stop - sl.start
                    nc.vector.tensor_mul(
                        A[:, sl],
                        p01[:, sl, None, :].broadcast_to((P, n, 16, 16)),
                        p23[:, sl, :, None].broadcast_to((P, n, 16, 16)))
                return A

            # XBAR issues go out before any Cs copy is queued on ACT: SC0's
            # transposes issue from SP, SC1's from ACT so the two halves of
            # the transpose work overlap instead of serializing on one queue.
            def xbars(eng, A):
                Af = A[:].rearrange("p i b a -> p i (b a)")
                at2 = atp.tile([P, 2 * SCM, P], F16, tag="at")
                for h in range(2):
                    eng.dma_start_transpose(
                        out=at2[:, h * SCM:(h + 1) * SCM, :],
                        in_=Af[:, h * 4:(h + 1) * 4, :].rearrange(
                            "p i n -> p (i n)"))
                return at2

            # ---- per-super-chunk back: A -> C -> out rows ----
            def back(sc, at2, p45):
                O = work.tile([P, SCM, 10], F32, tag="O")
                Ds = work.tile([P, SCM, 10, 16], F16, tag="Ds")
                T1 = work.tile([P, SCM, 10, 8], F16, tag="T1")
                for g in range(2):          # groups of 4 chunks
                    cpp = ps_c.tile([P, 4, 256], F32, tag="cp")
                    for j in range(4):
                        i = g * 4 + j
                        for k in range(2):
                            nc.tensor.matmul(cpp[:, j, 0:160],
                                             lhsT=at2[:, 2 * i + k, :],
                                             rhs=s2_sb[:, k, :],
                                             start=(k == 0), stop=(k == 1))
                    Cs = work.tile([P, 4, 10, 16], F16, tag=f"Cs{g}")
                    nc.scalar.copy(
                        out=Cs[:].rearrange("p j c v -> p j (c v)"),
                        in_=cpp[:, :, 0:160])
                    sl = slice(g * 4, (g + 1) * 4)
                    nc.vector.tensor_mul(
                        Ds[:, sl], Cs[:],
                        p45[:, sl, None, :].broadcast_to((P, 4, 10, 16)))
                    nc.vector.tensor_add(T1[:, sl], Ds[:, sl, :, 0:8],
                                         Ds[:, sl, :, 8:16])
                # v-contraction as an add tree (packed fp16 -> HW 2x mode)
                T2 = work.tile([P, SCM, 10, 4], F16, tag="T2")
                nc.vector.tensor_add(T2[:], T1[:, :, :, 0:4], T1[:, :, :, 4:8])
                T3 = work.tile([P, SCM, 10, 2], F16, tag="T3")
                nc.vector.tensor_add(T3[:], T2[:, :, :, 0:2], T2[:, :, :, 2:4])
                nc.vector.tensor_add(O[:], T3[:, :, :, 0], T3[:, :, :, 1])
                nc.scalar.dma_start(
                    out=od[:].rearrange("p (i c) -> p i c", i=M)[
                        :, sc * SCM:(sc + 1) * SCM, :],
                    in_=O[:])

            E0 = front_pre(0)
            E1 = front_pre(1)
            p010, p230, p450 = front(0, E0)
            A0 = build_a(p010, p230)
            p011, p231, p451 = front(1, E1)
            A1 = build_a(p011, p231)
            at20 = xbars(nc.sync, A0)
            at21 = xbars(nc.sync, A1)
            back(0, at20, p450)
            back(1, at21, p451)
    nc.compile()
    return nc


_CACHE = {}


def _host_prep(x, cuts, leaf_score, temperature):
    x = np.ascontiguousarray(np.asarray(x, dtype=np.float32))
    cuts = np.asarray(cuts, dtype=np.float32)
    leaf_score = np.asarray(leaf_score, dtype=np.float32)
    invt = 1.0 / float(np.asarray(temperature).reshape(-1)[0])

    sc = np.sort(cuts, axis=1)
    bias = np.cumsum(np.concatenate([np.zeros((6, 1), np.float32), -sc],
                                    axis=1, dtype=np.float32), axis=1)  # [6,4]
    W = np.arange(1.0, 5.0, dtype=np.float32)
    w2 = np.tile(((W - 1.0) * invt)[None, :], (6, 1))                   # [6,4]
    bt = bias * invt                                                    # [6,4]
    cst = np.ascontiguousarray(np.broadcast_to(
        np.stack([w2, bt]).reshape(1, 48), (P, 48)).astype(np.float32))
    # S2[w,(c,v)] = leaf_score[w*16+v, c], then permute rows a*16+b -> b*16+a
    # to match the device A[i, b, a] memory order.
    s2 = (leaf_score.reshape(256, 16, 10).transpose(0, 2, 1)
          .reshape(16, 16, 160).transpose(1, 0, 2).reshape(256, 160))
    s2 = np.ascontiguousarray(s2.astype(np.float16))

    xs = x.reshape(N_CORES, P, M * 6)
    in_maps = [{"x": xs[i], "cst": cst, "s2": s2} for i in range(N_CORES)]
    return invt, in_maps


def kernel(x, cuts, leaf_score, temperature):
    invt, in_maps = _host_prep(x, cuts, leaf_score, temperature)
    key = ("nc", float(invt))
    if key not in _CACHE:
        _CACHE[key] = _build_nc(invt)
        _CACHE["nc"] = _CACHE[key]
    nc = _CACHE[key]
    res = run_bass_kernel_spmd(nc, in_maps, list(range(N_CORES))).results
    out = np.concatenate([r["o"].reshape(BC, 10) for r in res], axis=0)
    return out.astype(np.float32)


# revision 24
# speedup vs baseline: 1.0038x; 1.0038x over previous
"""DNDT (deep neural decision tree) forward kernel for 8 Trainium2 NeuronCores.

Math (per batch row b of 16384):
  h[f,j]   = (x[b,f] * W[j] + bias[f,j]) / t,  W = [1..4], bias = cumsum([0,-sorted_cuts])
  bins     = softmax_j(h)                       # [6, 4]
  leaf     = kron(bins[0], ..., bins[5])        # [4096]
  out[b]   = leaf @ leaf_score                  # [10]

Device algorithm (pure data parallel, 2048 rows/core, batch-major layout
[128 partitions x 16 rows-per-partition], fp16 datapath after the exp):
  * softmax shift g(x) = (x + 3*relu(x))/t keeps every exponent <= 0, so
    E = exp(h - g) never overflows; exp runs on the scalar (ACT) engine.
  * bins are normalized PER FEATURE (En = E / sum_j E) right after the exp:
    all kron products then live in [0,1], which makes the fp16 pipeline safe.
  * leaf is factored 4+2: A = p01 (x) p23 (256-wide, [i, b, a] memory order
    with a host-side S2 row permutation), p45 (16-wide);
      C[b,(c,v)] = A @ S2,  S2[u,(c,v)] = leaf_score[u*16+v, c]   (PE, fp16)
      out[b,c]   = sum_v C[b,c,v] * p45[b,v]            (DVE mult + add tree)
  * A is transposed for the matmul by the DMA XBAR (dma_start_transpose,
    SBUF->SBUF fp16) - no PE transposes, no PSUM bounce copies.  The XBARs
    run on the DMA engines and overlap the DVE front.
  * C is copied out of PSUM by ACT in fp16; the v-contraction is a DVE mult
    plus a packed fp16 add-tree (the only shapes the HW runs in 2x mode).
  * output DMAs issue from ACT so the XBARs (on SP) never block them; the
    input DMA order (cst, then x per-super-chunk, then S2) unblocks the DVE
    front as early as possible.
"""

import numpy as np

import concourse.bass as bass
import concourse.tile as tile
from concourse import bacc, mybir
from concourse.bass_utils import run_bass_kernel_spmd

N_CORES = 8
B = 16384
BC = B // N_CORES          # rows per core = 2048
P = 128                    # partitions
M = BC // P                # rows per partition = 16
NSC = 2                    # super-chunks (pipeline stages)
SCM = M // NSC             # rows per partition per super-chunk = 8
F32 = mybir.dt.float32
F16 = mybir.dt.float16
AX = mybir.AxisListType
OP = mybir.AluOpType


def _build_nc(invt):
    neg3invt = -3.0 * invt
    nc = bacc.Bacc("TRN2", target_bir_lowering=False, debug=False,
                   num_devices=N_CORES)
    xd = nc.dram_tensor("x", [P, M * 6], F32, kind="ExternalInput")
    cstd = nc.dram_tensor("cst", [P, 48], F32, kind="ExternalInput")
    s2d = nc.dram_tensor("s2", [256, 160], F16, kind="ExternalInput")
    od = nc.dram_tensor("o", [P, M * 10], F32, kind="ExternalOutput")

    with tile.TileContext(nc) as tc:
        with tc.tile_pool(name="consts", bufs=1) as consts, \
             tc.tile_pool(name="work", bufs=2) as work, \
             tc.tile_pool(name="atp", bufs=2) as atp, \
             tc.tile_pool(name="ps_c", bufs=4, space="PSUM") as ps_c:
            cst_st = consts.tile([P, 2, 6, 4], F32)
            nc.sync.dma_start(out=cst_st[:].rearrange("p k f j -> p (k f j)"),
                              in_=cstd[:])
            x_st = consts.tile([P, M, 6], F32)
            xdv = xd[:].rearrange("p (i f) -> p i f", i=M)
            for sc in range(NSC):
                sl = slice(sc * SCM, (sc + 1) * SCM)
                nc.sync.dma_start(out=x_st[:, sl], in_=xdv[:, sl])
            s2_sb = consts.tile([P, 2, 160], F16)
            nc.sync.dma_start(out=s2_sb[:],
                              in_=s2d[:].rearrange("(k p) n -> p k n", p=P))

            # ---- per-super-chunk front: x -> En (normalized bins, fp16) ----
            # split pre-exp / post-exp so both SCs' H chains queue first and
            # fill the DVE bubble while ACT runs the exps; SC0's post-exp and
            # A then complete as early as possible to start the XBAR chain.
            def front_pre(sc):
                xv = x_st[:, sc * SCM:(sc + 1) * SCM, :]
                r2 = work.tile([P, SCM, 6, 1], F32, tag="r2")
                nc.vector.tensor_scalar(out=r2[:, :, :, 0], in0=xv,
                                        scalar1=0.0, scalar2=neg3invt,
                                        op0=OP.max, op1=OP.mult)
                Hm = work.tile([P, SCM, 6, 4], F32, tag="Hm")
                nc.vector.tensor_mul(
                    Hm[:], xv[:, :, :, None].broadcast_to((P, SCM, 6, 4)),
                    cst_st[:, 0:1, :, :].broadcast_to((P, SCM, 6, 4)))
                G = work.tile([P, SCM, 6, 4], F32, tag="G")
                nc.vector.tensor_add(
                    G[:], r2[:].broadcast_to((P, SCM, 6, 4)),
                    cst_st[:, 1:2, :, :].broadcast_to((P, SCM, 6, 4)))
                H = work.tile([P, SCM, 6, 4], F32, tag="H")
                nc.vector.tensor_add(H[:], Hm[:], G[:])
                E = work.tile([P, SCM, 6, 4], F32, tag="E")
                nc.scalar.activation(E[:].rearrange("p i f j -> p (i f j)"),
                                     H[:].rearrange("p i f j -> p (i f j)"),
                                     mybir.ActivationFunctionType.Exp)
                return E

            def front(sc, E):
                Z = work.tile([P, SCM, 6], F32, tag="Z")
                nc.vector.tensor_reduce(Z[:], E[:], axis=AX.X, op=OP.add)
                zrf = work.tile([P, SCM, 6], F32, tag="zrf")
                nc.vector.reciprocal(zrf[:], Z[:])
                En = work.tile([P, SCM, 6, 4], F16, tag="En")
                nc.vector.tensor_mul(
                    En[:], E[:], zrf[:, :, :, None].broadcast_to((P, SCM, 6, 4)))

                p01 = work.tile([P, SCM, 16], F16, tag="p01")
                nc.vector.tensor_mul(
                    p01[:].rearrange("p i (a b) -> p i a b", a=4),
                    En[:, :, 0, :, None].broadcast_to((P, SCM, 4, 4)),
                    En[:, :, 1, None, :].broadcast_to((P, SCM, 4, 4)))
                p23 = work.tile([P, SCM, 16], F16, tag="p23")
                nc.vector.tensor_mul(
                    p23[:].rearrange("p i (a b) -> p i a b", a=4),
                    En[:, :, 2, :, None].broadcast_to((P, SCM, 4, 4)),
                    En[:, :, 3, None, :].broadcast_to((P, SCM, 4, 4)))
                p45 = work.tile([P, SCM, 16], F16, tag="p45")
                nc.vector.tensor_mul(
                    p45[:].rearrange("p i (a b) -> p i a b", a=4),
                    En[:, :, 4, :, None].broadcast_to((P, SCM, 4, 4)),
                    En[:, :, 5, None, :].broadcast_to((P, SCM, 4, 4)))

                return p01, p23, p45

            # A[i, b, a] = p01[i, a] * p23[i, b]: two DVE instructions
            # aligned with the XBAR halves (single producer each).
            def build_a(p01, p23):
                A = work.tile([P, SCM, 16, 16], F16, tag="A")
                for sl in (slice(0, 4), slice(4, 8)):
                    n = sl.stop - sl.start
                    nc.vector.tensor_mul(
                        A[:, sl],
                        p01[:, sl, None, :].broadcast_to((P, n, 16, 16)),
                        p23[:, sl, :, None].broadcast_to((P, n, 16, 16)))
                return A

            # XBAR issues go out before any Cs copy is queued on ACT: SC0's
            # transposes issue from SP, SC1's from ACT so the two halves of
            # the transpose work overlap instead of serializing on one queue.
            def xbars(eng, A):
                Af = A[:].rearrange("p i b a -> p i (b a)")
                at2 = atp.tile([P, 2 * SCM, P], F16, tag="at")
                for h in range(2):
                    eng.dma_start_transpose(
                        out=at2[:, h * SCM:(h + 1) * SCM, :],
                        in_=Af[:, h * 4:(h + 1) * 4, :].rearrange(
                            "p i n -> p (i n)"))
                return at2

            # ---- per-super-chunk back: A -> C -> out rows ----
            def back(sc, at2, p45):
                O = work.tile([P, SCM, 10], F32, tag="O")
                Ds = work.tile([P, SCM, 10, 16], F16, tag="Ds")
                T1 = work.tile([P, SCM, 10, 8], F16, tag="T1")
                T2 = work.tile([P, SCM, 10, 4], F16, tag="T2")
                T3 = work.tile([P, SCM, 10, 2], F16, tag="T3")
                for g in range(2):          # groups of 4 chunks
                    cpp = ps_c.tile([P, 4, 256], F32, tag="cp")
                    for j in range(4):
                        i = g * 4 + j
                        for k in range(2):
                            nc.tensor.matmul(cpp[:, j, 0:160],
                                             lhsT=at2[:, 2 * i + k, :],
                                             rhs=s2_sb[:, k, :],
                                             start=(k == 0), stop=(k == 1))
                    Cs = work.tile([P, 4, 10, 16], F16, tag=f"Cs{g}")
                    nc.scalar.copy(
                        out=Cs[:].rearrange("p j c v -> p j (c v)"),
                        in_=cpp[:, :, 0:160])
                    sl = slice(g * 4, (g + 1) * 4)
                    nc.vector.tensor_mul(
                        Ds[:, sl], Cs[:],
                        p45[:, sl, None, :].broadcast_to((P, 4, 10, 16)))
                    # v-contraction finishes per group (packed fp16 add tree
                    # in HW 2x mode) so the chain after the last D-mult and
                    # the final output DMA are as short as possible
                    nc.vector.tensor_add(T1[:, sl], Ds[:, sl, :, 0:8],
                                         Ds[:, sl, :, 8:16])
                    nc.vector.tensor_add(T2[:, sl], T1[:, sl, :, 0:4],
                                         T1[:, sl, :, 4:8])
                    nc.vector.tensor_add(T3[:, sl], T2[:, sl, :, 0:2],
                                         T2[:, sl, :, 2:4])
                    nc.vector.tensor_add(O[:, sl], T3[:, sl, :, 0],
                                         T3[:, sl, :, 1])
                    nc.scalar.dma_start(
                        out=od[:].rearrange("p (i c) -> p i c", i=M)[
                            :, sc * SCM + g * 4:sc * SCM + (g + 1) * 4, :],
                        in_=O[:, sl])

            E0 = front_pre(0)
            E1 = front_pre(1)
            p010, p230, p450 = front(0, E0)
            A0 = build_a(p010, p230)
            p011, p231, p451 = front(1, E1)
            A1 = build_a(p011, p231)
            at20 = xbars(nc.sync, A0)
            at21 = xbars(nc.sync, A1)
            back(0, at20, p450)
            back(1, at21, p451)
    nc.compile()
    return nc


_CACHE = {}


def _host_prep(x, cuts, leaf_score, temperature):
    x = np.ascontiguousarray(np.asarray(x, dtype=np.float32))
    cuts = np.asarray(cuts, dtype=np.float32)
    leaf_score = np.asarray(leaf_score, dtype=np.float32)
    invt = 1.0 / float(np.asarray(temperature).reshape(-1)[0])

    sc = np.sort(cuts, axis=1)
    bias = np.cumsum(np.concatenate([np.zeros((6, 1), np.float32), -sc],
                                    axis=1, dtype=np.float32), axis=1)  # [6,4]
    W = np.arange(1.0, 5.0, dtype=np.float32)
    w2 = np.tile(((W - 1.0) * invt)[None, :], (6, 1))                   # [6,4]
    bt = bias * invt                                                    # [6,4]
    cst = np.ascontiguousarray(np.broadcast_to(
        np.stack([w2, bt]).reshape(1, 48), (P, 48)).astype(np.float32))
    # S2[w,(c,v)] = leaf_score[w*16+v, c], then permute rows a*16+b -> b*16+a
    # to match the device A[i, b, a] memory order.
    s2 = (leaf_score.reshape(256, 16, 10).transpose(0, 2, 1)
          .reshape(16, 16, 160).transpose(1, 0, 2).reshape(256, 160))
    s2 = np.ascontiguousarray(s2.astype(np.float16))

    xs = x.reshape(N_CORES, P, M * 6)
    in_maps = [{"x": xs[i], "cst": cst, "s2": s2} for i in range(N_CORES)]
    return invt, in_maps


def kernel(x, cuts, leaf_score, temperature):
    invt, in_maps = _host_prep(x, cuts, leaf_score, temperature)
    key = ("nc", float(invt))
    if key not in _CACHE:
        _CACHE[key] = _build_nc(invt)
        _CACHE["nc"] = _CACHE[key]
    nc = _CACHE[key]
    res = run_bass_kernel_spmd(nc, in_maps, list(range(N_CORES))).results
    out = np.concatenate([r["o"].reshape(BC, 10) for r in res], axis=0)
    return out.astype(np.float32)
